# revision 23
# baseline (speedup 1.0000x reference)
"""Trainium2 Bass kernel for nn_CapsuleLayer (B=32, In=128, Din=256, ch=32, Nc=47, Dc=64).

Sharding: over the OUTPUT-CAPSULE axis Nc (47 -> pad 48 = 8 cores x 6 capsules).
W (94 MiB) is the dominant HBM tensor -- Nc-sharding reads W exactly once total.

bf16 pipeline (rel_err ~6e-3 vs 2e-2 gate):
- stream (x|W) in bf16, partition-major HBM layout -> 32KB-contiguous DMA runs
- inputs_hat via bf16 matmuls (1 cy/row vs fp32's 4)
- IH stored TWICE from PSUM: k-inner [p,(c,n,k)] for the a-step and c-inner
  [p,(n,k,c)] for the s-step, so both big DVE muls hit the 2x bf16 perf mode
  (packed innermost operands; measured 0.64 ns/col vs 1.28 broadcast/1x)
- reductions as pairwise bf16 tree-adds (2x) instead of TENSOR_REDUCE (1x)

Routing iteration t (per core, Nsh=6 capsules):
  TMP  = IH * OUTr            (DVE 2x, k-inner)
  A    = tree-fold k 64->1    (DVE 2x, last level fp32)
  E    = exp(sum_t A)         (ACT, written transposed to [p,(n,c)])
  Zp   = reduce_c E           (DVE, into SCRATCH[384:390])
  TMP2 = IHC * E              (DVE 2x, c-inner)
  P2   = tree-fold c 32->1    (DVE 2x, into SCRATCH[0:384])
  pS   = BD4^T [P2|Zp]        (PE partition reduce over (b,rr))
  S    = pS/Z + Brep ; OUT = squash(S)  (small [32,384] ops)
Iteration 1 (uniform c): S1 = psum_s1/IN + Brep via PSUM-accumulated
BD4^T IH_c matmuls during phase 1.

Toolchain constraint: EVERY engine instruction accepts at most ONE sync wait
at codegen.  Same-engine deps are free (program order / one monotonic sem per
engine); cross-engine fan-in is handled by absorb ops (tiny reads that
pre-observe a sem) and dummy matmuls on the PE.
"""

import numpy as np

B, IN, DIN = 32, 128, 256
CH, NC, DC = 32, 47, 64
NCP = 48          # padded Nc
NSH = 6           # capsules per core
NCORES = 8
NK = NSH * DC     # 384
EPS = 1e-7

_cache = {}


def _build_nc():
    import concourse.bass as bass
    import concourse.tile as tile
    from concourse import mybir
    from concourse.tile_rust import add_dep_helper

    f32 = mybir.dt.float32
    bf = mybir.dt.bfloat16
    nc = bass.Bass()

    # partition-major packed stream: xw[d, cd, 0:128]=xT, [128:512]=wT (bf16)
    xw = nc.dram_tensor("xw", [128, CH * 2, 512], bf, kind="ExternalInput")
    # consts: [bd4(0:32) | bd4t(rows0:32, 32:160) | brep(rows0:32, 160:544)]
    cst = nc.dram_tensor("cst", [128, 544], bf, kind="ExternalInput")
    out_d = nc.dram_tensor("out", [B, NK], f32, kind="ExternalOutput")

    ADD = mybir.AluOpType.add
    MULT = mybir.AluOpType.mult
    AX = mybir.AxisListType.X
    AF = mybir.ActivationFunctionType

    with tile.TileContext(nc) as tc:
        with (
            tc.tile_pool(name="singles", bufs=1) as singles,
            tc.tile_pool(name="work", bufs=1) as work,
            tc.tile_pool(name="small", bufs=2) as small,
            tc.tile_pool(name="ps_ih", bufs=3, space="PSUM") as ps_ih,
            tc.tile_pool(name="ps_s1", bufs=1, space="PSUM") as ps_s1,
            tc.tile_pool(name="ps_s", bufs=2, space="PSUM") as ps_s,
            tc.tile_pool(name="ps_rep", bufs=2, space="PSUM") as ps_rep,
        ):
            cst_t = singles.tile([128, 544], bf)
            c_dma = nc.sync.dma_start(out=cst_t[:], in_=cst[:])
            bd4_t = cst_t[:, 0:B]                 # [128, 32] bf16
            bd4t_t = cst_t[0:B, B:B + 128]        # [32, 128] bf16
            brep_t = cst_t[0:B, B + 128:B + 128 + NK]   # [32, 384] bf16
            eps_t = singles.tile([B, 1], f32)
            nc.vector.memset(eps_t[:], EPS)
            # DVE/ACT pre-observe the const-DMA sem
            dve_scratch = singles.tile([4, 8], bf)
            nc.vector.tensor_copy(dve_scratch[:2, 0:2], cst_t[:2, :2])
            act_scratch = singles.tile([4, 8], bf)
            nc.scalar.copy(act_scratch[:2, 0:2], cst_t[:2, :2])
            act_f32 = singles.tile([4, 2], f32)
            nc.scalar.activation(act_f32[:2, 0:2], act_scratch[:2, 0:2],
                                 AF.Exp)

            IH = singles.tile([128, CH, NK], bf)      # k-inner
            IHC = singles.tile([128, NK, CH + 1], bf)  # c-inner, pad stride 33
            STREAM = singles.tile([128, CH * 2, 512], bf)
            TMP = singles.tile([128, CH * NK], bf)    # mul product scratch
            U1 = singles.tile([128, 6144], bf)
            U2 = singles.tile([128, 3072], bf)
            SCR = singles.tile([128, NK + NSH], bf)   # [P2 | Zp]
            A2 = singles.tile([128, CH * NSH], f32)
            A3 = singles.tile([128, CH * NSH], f32)
            E = singles.tile([128, NSH * CH], bf)     # [p, (n, c)]
            OUTr = singles.tile([128, NK], bf)
            GPC = 6                                   # gpsimd channel share
            NGP = 1                                   # gpsimd capsule share
            TMPG = singles.tile([128, GPC * NK], bf)
            UG1 = singles.tile([128, GPC * NSH * 32], bf)
            UG2 = singles.tile([128, GPC * NSH * 16], bf)
            TMP2G = singles.tile([128, NGP * DC * CH], bf)
            VG1 = singles.tile([128, NGP * DC * 16], bf)
            VG2 = singles.tile([128, NGP * DC * 8], bf)
            VGF = singles.tile([128, NGP * DC], bf)

            # Absorb the const-DMA sem into the PE clock (PE nop).
            last_dummy = nc.tensor.nop()
            add_dep_helper(last_dummy.ins, c_dma.ins, sync=True,
                           reason="absorb cst DMA sem into PE clock")

            psum_s1 = ps_s1.tile([B, NK], f32)

            # ---------------- phase 1: inputs_hat + iter-1 s ----------------
            s_dmas = []
            dma_splits = [(0, 2), (2, 22), (22, 43), (43, 64)]
            for gi, (lo, hi) in enumerate(dma_splits):
                dd = nc.sync.dma_start(
                    out=STREAM[:, lo:hi, :],
                    in_=xw[:, lo:hi, :],
                )
                if gi > 0:
                    add_dep_helper(dd.ins, s_dmas[0].ins, sync=True,
                                   reason="first chunk gets full DMA bandwidth")
                s_dmas.append(dd)
            # channel processing order (c, c+16) interleaved so the iter-1
            # tree-fold over c can start mid-phase (chunk j needs channels
            # 4j..4j+3 and 16+4j..19+4j = the first 8(j+1) positions)
            ch_order = []
            for t in range(CH // 2):
                ch_order += [t, t + CH // 2]
            U1s = U1[:].rearrange("p (n k c) -> p n k c", n=NSH, k=DC)

            def s1_mm(pos):
                nc.tensor.matmul(
                    psum_s1[:], bd4_t[:], IH[:, ch_order[pos], :],
                    start=(pos == 0), stop=(pos == CH - 1),
                    skip_group_check=True,
                )

            copy_last = []      # last psum reader per position
            for pos, c in enumerate(ch_order):
                if pos >= 3:
                    # absorb the psum-slot WAR tick into the PE clock
                    dmy = nc.tensor.nop()
                    add_dep_helper(dmy.ins, copy_last[pos - 3].ins, sync=True,
                                   reason="absorb psum WAR tick on PE")
                    last_dummy = dmy
                psum_ih = ps_ih.tile([128, NK], f32, tag="ih")
                for dc in range(2):
                    cd = pos * 2 + dc
                    mih = nc.tensor.matmul(
                        psum_ih[:], STREAM[:, cd, 0:128], STREAM[:, cd, 128:512],
                        start=(dc == 0), stop=(dc == 1),
                    )
                    if dc == 0:
                        add_dep_helper(mih.ins, last_dummy.ins, sync=False,
                                       reason="order dummy before matmul")
                # both copies for channel c on ONE engine (alternating) so
                # the WAR absorb needs a single sem
                if pos % 2 == 0:
                    nc.vector.tensor_copy(IH[:, c, :], psum_ih[:])
                    cl = nc.vector.tensor_copy(IHC[:, :, c], psum_ih[:])
                else:
                    nc.scalar.copy(IH[:, c, :], psum_ih[:])
                    cl = nc.scalar.copy(IHC[:, :, c], psum_ih[:])
                copy_last.append(cl)
                # iter-1 s accumulation, lagged 4 positions behind the copies
                if pos >= 4:
                    s1_mm(pos - 4)
            for pp in range(CH - 4, CH):
                s1_mm(pp)

            _absn = [0]

            def absorb(eng, src_ap):
                """Tiny copy on `eng` reading src_ap: pre-observes the
                producer's sem so the next real op keeps a single wait."""
                _absn[0] += 1
                scr = small.tile([2, 2], f32, tag="abs%d" % _absn[0])
                if eng == "v":
                    return nc.vector.tensor_copy(scr[:], src_ap)
                return nc.scalar.copy(scr[:], src_ap)

            def squash(S, it):
                """S: [B, NK] f32 sbuf tile -> OUT tile (bf16 it<3, f32 it=3)."""
                Ssq = work.tile([B, NK], f32, tag="Su")
                nc.vector.tensor_mul(Ssq[:], S[:], S[:])
                m2 = small.tile([B, NSH], f32, tag="m2")
                nc.vector.tensor_reduce(
                    m2[:], Ssq[:].rearrange("p (n k) -> p n k", n=NSH),
                    axis=AX, op=ADD,
                )
                d1 = small.tile([B, NSH], f32, tag="d1")
                nc.vector.tensor_scalar_add(d1[:], m2[:], 1.0)
                rd1 = small.tile([B, NSH], f32, tag="rd1")
                nc.vector.reciprocal(rd1[:], d1[:])
                absorb("s", m2[:2, :2])          # ACT clock <- m2 (DVE)
                # rsqrt(m2+eps) = exp(-0.5*ln(m2+eps)); ln+exp share one
                # ACT table set (no SQRT table thrash)
                ln_ = small.tile([B, NSH], f32, tag="ln")
                nc.scalar.activation(ln_[:], m2[:], AF.Ln, bias=eps_t[:])
                rsq = small.tile([B, NSH], f32, tag="rsq")
                nc.scalar.activation(rsq[:], ln_[:], AF.Exp, scale=-0.5)
                absorb("v", rsq[:2, :2])         # DVE clock <- rsq (ACT)
                t_ = small.tile([B, NSH], f32, tag="t")
                nc.vector.tensor_mul(t_[:], m2[:], rsq[:])
                g_ = small.tile([B, NSH], f32, tag="g")
                nc.vector.tensor_mul(g_[:], t_[:], rd1[:])
                OUT = work.tile([B, NK], f32 if it == 3 else bf,
                                tag="out%d" % it)
                nc.vector.tensor_mul(
                    OUT[:].rearrange("p (n k) -> p n k", n=NSH),
                    S[:].rearrange("p (n k) -> p n k", n=NSH),
                    g_[:].rearrange("p (n o) -> p n o", o=1)
                        .broadcast_to([B, NSH, DC]),
                )
                return OUT

            rep_mm_prev = [None]
            mm_last_ref = [None]

            def replicate(OUTb, it):
                """OUTb [B, NK] bf16 -> OUTr [128, NK] bf16 (row b -> 4b..4b+3)."""
                pr = ps_rep.tile([128, NK], f32, tag="rep")
                mm = nc.tensor.matmul(pr[:], bd4t_t[:], OUTb[:],
                                      start=True, stop=True)
                rep_mm_prev[0] = mm
                if it == 2:
                    # DVE observes gp's mul1 (OUTr reader) before rewriting OUTr
                    gscr2 = small.tile([2, 2], bf, tag="gor")
                    ga = nc.vector.tensor_copy(gscr2[:], TMPG[:2, 0:2])
                    cp = nc.vector.tensor_copy(OUTr[:], pr[:])
                    add_dep_helper(cp.ins, ga.ins, sync=False,
                                   reason="gp OUTr read absorbed first")
                else:
                    cp = nc.vector.tensor_copy(OUTr[:], pr[:])
                return mm, cp

            # ---------------- iter 1 ----------------
            S1 = work.tile([B, NK], f32, tag="S")
            nc.vector.scalar_tensor_tensor(
                out=S1[:], in0=psum_s1[:], scalar=1.0 / IN, in1=brep_t[:],
                op0=MULT, op1=ADD,
            )
            OUT1 = squash(S1, 1)
            rep_mm, rep_cp = replicate(OUT1, 1)

            TMPk = TMP[:].rearrange("p (c n k) -> p c n k", c=CH, n=NSH)
            TMPc = TMP[:].rearrange("p (n k c) -> p n k c", n=NSH, k=DC)
            U1k = U1[:].rearrange("p (c n k) -> p c n k", c=CH, n=NSH)
            U2k = U2[:].rearrange("p (c n k) -> p c n k", c=CH, n=NSH)
            U1c = U1[:].rearrange("p (n k c) -> p n k c", n=NSH, k=DC)
            U2c = U2[:].rearrange("p (n k c) -> p n k c", n=NSH, k=DC)

            # gpsimd pre-observes the ACT-side IH copies once
            gwarm = small.tile([2, 2], bf, tag="gwarm")
            gw = nc.gpsimd.tensor_copy(gwarm[:], IH[:2, ch_order[-1], 0:2])

            for it in (2, 3):
                DC_ = CH - GPC                        # DVE channel count
                DN = NSH - NGP                        # DVE capsule count
                # ---- a-step: TMP = IH * OUTr ; A = tree-fold k ----
                # gpsimd slice: channels DC_..CH
                mg = nc.gpsimd.tensor_mul(
                    TMPG[:].rearrange("p (c nk) -> p c nk", c=GPC),
                    IH[:, DC_:CH, :],
                    OUTr[:].rearrange("p (o nk) -> p o nk", o=1)
                          .broadcast_to([128, GPC, NK]),
                )
                if it == 2:
                    add_dep_helper(mg.ins, gw.ins, sync=False,
                                   reason="warmup before first gp mul")
                TG = TMPG[:].rearrange("p (c n k) -> p c n k", c=GPC, n=NSH)
                G1 = UG1[:].rearrange("p (c n k) -> p c n k", c=GPC, n=NSH)
                G2 = UG2[:].rearrange("p (c n k) -> p c n k", c=GPC, n=NSH)
                nc.gpsimd.tensor_add(G1[:, :, :, 0:32], TG[:, :, :, 0:32],
                                     TG[:, :, :, 32:64])
                nc.gpsimd.tensor_add(G2[:, :, :, 0:16], G1[:, :, :, 0:16],
                                     G1[:, :, :, 16:32])
                nc.gpsimd.tensor_add(G1[:, :, :, 0:8], G2[:, :, :, 0:8],
                                     G2[:, :, :, 8:16])
                nc.gpsimd.tensor_add(G2[:, :, :, 0:4], G1[:, :, :, 0:4],
                                     G1[:, :, :, 4:8])
                nc.gpsimd.tensor_add(G1[:, :, :, 0:2], G2[:, :, :, 0:2],
                                     G2[:, :, :, 2:4])
                At = A2 if it == 2 else A3
                gp_a = nc.gpsimd.tensor_add(
                    At[:, DC_ * NSH:].rearrange("p (c n o) -> p c n o",
                                                c=GPC, o=1),
                    G1[:, :, :, 0:1], G1[:, :, :, 1:2],
                )
                # DVE slice: channels 0..DC_
                nc.vector.tensor_mul(
                    TMP[:].rearrange("p (c nk) -> p c nk", c=CH)[:, 0:DC_, :],
                    IH[:, 0:DC_, :],
                    OUTr[:].rearrange("p (o nk) -> p o nk", o=1)
                          .broadcast_to([128, DC_, NK]),
                )
                nc.vector.tensor_add(U1k[:, 0:DC_, :, 0:32],
                                     TMPk[:, 0:DC_, :, 0:32],
                                     TMPk[:, 0:DC_, :, 32:64])
                nc.vector.tensor_add(U2k[:, 0:DC_, :, 0:16],
                                     U1k[:, 0:DC_, :, 0:16],
                                     U1k[:, 0:DC_, :, 16:32])
                nc.vector.tensor_add(U1k[:, 0:DC_, :, 0:8],
                                     U2k[:, 0:DC_, :, 0:8],
                                     U2k[:, 0:DC_, :, 8:16])
                nc.vector.tensor_add(U2k[:, 0:DC_, :, 0:4],
                                     U1k[:, 0:DC_, :, 0:4],
                                     U1k[:, 0:DC_, :, 4:8])
                nc.vector.tensor_add(U1k[:, 0:DC_, :, 0:2],
                                     U2k[:, 0:DC_, :, 0:2],
                                     U2k[:, 0:DC_, :, 2:4])
                nc.vector.tensor_add(
                    At[:, 0:DC_ * NSH].rearrange("p (c n o) -> p c n o",
                                                 c=DC_, o=1),
                    U1k[:, 0:DC_, :, 0:1], U1k[:, 0:DC_, :, 1:2],
                )
                if it == 2:
                    BL = A2
                else:
                    BL = A3
                    absorb("v", A3[:2, DC_ * NSH:DC_ * NSH + 2])
                    nc.vector.tensor_add(A3[:], A3[:], A2[:])
                # ---- E = exp(BL), transposed write to [p, (n, c)] ----
                abs_scr = small.tile([2, 2], f32, tag="ag%d" % it)
                if it == 2:
                    ab1 = nc.scalar.copy(abs_scr[:],
                                         At[:2, DC_ * NSH:DC_ * NSH + 2])
                else:
                    # ACT observes gp's E-consumers of the previous iteration
                    # (WAR on E) and, transitively, gp's A-slice
                    ab1 = nc.scalar.copy(abs_scr[:], TMP2G[:2, 0:2])
                ab2 = absorb("s", BL[:2, :2])   # ACT clock <- DVE tree
                add_dep_helper(ab2.ins, ab1.ins, sync=False,
                               reason="order absorbs")
                exp_i = nc.scalar.activation(
                    E[:].rearrange("p (n c) -> p c n", n=NSH),
                    BL[:].rearrange("p (c n) -> p c n", c=CH),
                    AF.Exp,
                )
                add_dep_helper(exp_i.ins, ab2.ins, sync=False,
                               reason="absorbs before exp")
                # ---- Zp = sum_c E -> SCR[384:390] ----
                absorb("v", E[:2, :2])          # DVE clock <- E (ACT)
                with nc.allow_low_precision(reason="Z normalizer, positive sum"):
                    nc.vector.tensor_reduce(
                        SCR[:, NK:NK + NSH],
                        E[:].rearrange("p (n c) -> p n c", n=NSH),
                        axis=AX, op=ADD,
                    )
                # ---- s-step: TMP2 = IHC * E ; P2 = tree-fold c ----
                # gpsimd slice: capsule n = DN
                gscr = small.tile([2, 2], bf, tag="ge%d" % it)
                gsc = nc.gpsimd.tensor_copy(gscr[:], E[:2, :2])
                T2G = TMP2G[:].rearrange("p (k c) -> p k c", c=CH)
                m2g = nc.gpsimd.tensor_mul(
                    T2G,
                    IHC[:, DN * DC:NK, 0:CH],
                    E[:, DN * CH:].rearrange("p (o c) -> p o c", o=1)
                       .broadcast_to([128, DC, CH]),
                )
                add_dep_helper(m2g.ins, gsc.ins, sync=False,
                               reason="E absorb before gp mul2")
                W1 = VG1[:].rearrange("p (k c) -> p k c", c=16)
                W2 = VG2[:].rearrange("p (k c) -> p k c", c=8)
                nc.gpsimd.tensor_add(W1[:], T2G[:, :, 0:16], T2G[:, :, 16:32])
                nc.gpsimd.tensor_add(W2[:], W1[:, :, 0:8], W1[:, :, 8:16])
                nc.gpsimd.tensor_add(W1[:, :, 0:4], W2[:, :, 0:4],
                                     W2[:, :, 4:8])
                nc.gpsimd.tensor_add(W2[:, :, 0:2], W1[:, :, 0:2],
                                     W1[:, :, 2:4])
                gp_p = nc.gpsimd.tensor_add(
                    VGF[:].rearrange("p (k o) -> p k o", o=1),
                    W2[:, :, 0:1], W2[:, :, 1:2])
                # DVE slice: capsules 0..DN
                nc.vector.tensor_mul(
                    TMPc[:, 0:DN, :, :],
                    IHC[:, 0:DN * DC, 0:CH]
                       .rearrange("p (n k) c -> p n k c", n=DN),
                    E[:, 0:DN * CH].rearrange("p (n o c) -> p n o c", n=DN, o=1)
                       .broadcast_to([128, DN, DC, CH]),
                )
                nc.vector.tensor_add(U1c[:, 0:DN, :, 0:16],
                                     TMPc[:, 0:DN, :, 0:16],
                                     TMPc[:, 0:DN, :, 16:32])
                nc.vector.tensor_add(U2c[:, 0:DN, :, 0:8],
                                     U1c[:, 0:DN, :, 0:8],
                                     U1c[:, 0:DN, :, 8:16])
                nc.vector.tensor_add(U1c[:, 0:DN, :, 0:4],
                                     U2c[:, 0:DN, :, 0:4],
                                     U2c[:, 0:DN, :, 4:8])
                nc.vector.tensor_add(U2c[:, 0:DN, :, 0:2],
                                     U1c[:, 0:DN, :, 0:2],
                                     U1c[:, 0:DN, :, 2:4])
                nc.vector.tensor_add(
                    SCR[:, 0:DN * DC].rearrange("p (n k o) -> p n k o",
                                                n=DN, o=1),
                    U2c[:, 0:DN, :, 0:1], U2c[:, 0:DN, :, 1:2])
                # gp's P2 slice lands in SCR via DVE so the pS matmul has a
                # single (DVE) producer sem
                nc.vector.tensor_copy(SCR[:, DN * DC:NK], VGF[:])
                pS = ps_s.tile([B, NK + NSH], f32, tag="pS")
                mm_last = nc.tensor.matmul(pS[:], bd4_t[:], SCR[:],
                                           start=True, stop=True)
                mm_last_ref[0] = mm_last
                # ---- S = pS/Z + brep ----
                absorb("v", pS[:2, :2])         # DVE clock <- pS (PE)
                Rz = small.tile([B, NSH], f32, tag="Rz")
                nc.vector.reciprocal(Rz[:], pS[:, NK:NK + NSH])
                Su = work.tile([B, NK], f32, tag="Su2")
                nc.vector.tensor_mul(
                    Su[:].rearrange("p (n k) -> p n k", n=NSH),
                    pS[:, 0:NK].rearrange("p (n k) -> p n k", n=NSH),
                    Rz[:].rearrange("p (n o) -> p n o", o=1)
                        .broadcast_to([B, NSH, DC]),
                )
                S = work.tile([B, NK], f32, tag="S")
                nc.vector.tensor_add(S[:], Su[:], brep_t[:])
                OUT = squash(S, it)
                if it < 3:
                    rep_mm, rep_cp = replicate(OUT, it)
                else:
                    # absorb stream/cst DMA queue sems into SYNC first so the
                    # out-DMA's queue-reuse wait dedups to a single sem
                    for fin in (c_dma, *s_dmas):
                        fnop = nc.sync.nop()
                        add_dep_helper(fnop.ins, fin.ins, sync=True,
                                       reason="absorb DMA sem for queue reuse")
                    o_dma = nc.sync.dma_start(out=out_d[:], in_=OUT[:])
                    f_scr = small.tile([2, 4], f32, tag="fin")
                    f_act = nc.scalar.copy(f_scr[:, 0:2], OUT[:2, :2])
                    f_dve = nc.vector.tensor_copy(f_scr[:, 2:4], OUT[:2, :2])
                    for fin in (mm_last, f_act, f_dve, o_dma):
                        fnop = nc.sync.nop()
                        add_dep_helper(fnop.ins, fin.ins, sync=True,
                                       reason="absorb final sem for tail drain")

    return nc


def _pack_inputs(inputs, W, B_param):
    """Host-side shard + relayout. Returns list of 8 in_maps."""
    import ml_dtypes
    bf16 = ml_dtypes.bfloat16
    inputs = np.ascontiguousarray(inputs, dtype=np.float32)
    W = np.ascontiguousarray(W, dtype=np.float32)
    B_param = np.ascontiguousarray(B_param, dtype=np.float32)

    Wp = np.zeros((CH, NCP, DC, DIN), dtype=np.float32)
    Wp[:, :NC] = W
    Bp = np.zeros((NCP, DC), dtype=np.float32)
    Bp[:NC] = B_param

    # xt[(c,dc), dd, (b,rr)] = x[b, 4c+rr, 128dc+dd]
    x4 = inputs.reshape(B, CH, 4, 2, 128)           # b, c, rr, dc, dd
    xt = x4.transpose(1, 3, 4, 0, 2).reshape(CH * 2, 128, 128)
    bd4 = np.zeros((128, B), dtype=np.float32)
    bd4[np.arange(128), np.arange(128) // 4] = 1.0
    bd4t = bd4.T

    in_maps = []
    for core in range(NCORES):
        sl = slice(core * NSH, (core + 1) * NSH)
        Wc = Wp[:, sl]                               # c, n, k, d
        w5 = Wc.reshape(CH, NSH, DC, 2, 128)         # c n k dc dd
        wtc = w5.transpose(0, 3, 4, 1, 2).reshape(CH * 2, 128, NK)
        cstc = np.zeros((128, 544), dtype=np.float32)
        cstc[:, 0:B] = bd4
        cstc[0:B, B:B + 128] = bd4t
        cstc[0:B, B + 128:B + 128 + NK] = np.broadcast_to(
            Bp[sl].reshape(1, NK), (B, NK))
        xwc = np.concatenate([xt, wtc], axis=2)      # [64, 128, 512]
        # reorder chunks to the kernel's interleaved channel order
        ch_order = []
        for t in range(CH // 2):
            ch_order += [t, t + CH // 2]
        perm = np.zeros(CH * 2, dtype=np.int64)
        for p_, c_ in enumerate(ch_order):
            perm[2 * p_] = 2 * c_
            perm[2 * p_ + 1] = 2 * c_ + 1
        xwc = xwc[perm]
        # partition-major: [d, cd, 512] for 32KB-contiguous DMA runs
        xwc = np.ascontiguousarray(xwc.transpose(1, 0, 2)).astype(bf16)
        in_maps.append(dict(xw=xwc, cst=cstc.astype(bf16)))
    return in_maps


def _run(inputs, W, B_param, trace=False):
    from concourse.bass_utils import run_bass_kernel_spmd

    if "nc" not in _cache:
        _cache["nc"] = _build_nc()
    nc = _cache["nc"]
    in_maps = _pack_inputs(inputs, W, B_param)
    res = run_bass_kernel_spmd(nc, in_maps, core_ids=list(range(NCORES)),
                               trace=trace)
    outs = [r["out"].reshape(B, NSH, DC) for r in res.results]
    full = np.concatenate(outs, axis=1)[:, :NC, :]
    return np.ascontiguousarray(full.astype(np.float32)), res


def kernel(inputs, W, B_param):
    out, _ = _run(inputs, W, B_param, trace=False)
    return out


# revision 24
# speedup vs baseline: 1.2237x; 1.2237x over previous
"""Trainium2 Bass kernel for nn_CapsuleLayer (B=32, In=128, Din=256, ch=32, Nc=47, Dc=64).

Sharding: over the OUTPUT-CAPSULE axis Nc (47 -> pad 48 = 8 cores x 6 capsules).
W (94 MiB) is the dominant HBM tensor -- Nc-sharding reads W exactly once total.

bf16 pipeline (rel_err ~6e-3 vs 2e-2 gate):
- stream (x|W) in bf16, partition-major HBM layout -> 32KB-contiguous DMA runs
- inputs_hat via bf16 matmuls (1 cy/row vs fp32's 4)
- IH stored TWICE from PSUM: k-inner [p,(c,n,k)] for the a-step and c-inner
  [p,(n,k,c)] for the s-step, so both big DVE muls hit the 2x bf16 perf mode
  (packed innermost operands; measured 0.64 ns/col vs 1.28 broadcast/1x)
- reductions as pairwise bf16 tree-adds (2x) instead of TENSOR_REDUCE (1x)

Routing iteration t (per core, Nsh=6 capsules):
  TMP  = IH * OUTr            (DVE 2x, k-inner)
  A    = tree-fold k 64->1    (DVE 2x, last level fp32)
  E    = exp(sum_t A)         (ACT, written transposed to [p,(n,c)])
  Zp   = reduce_c E           (DVE, into SCRATCH[384:390])
  TMP2 = IHC * E              (DVE 2x, c-inner)
  P2   = tree-fold c 32->1    (DVE 2x, into SCRATCH[0:384])
  pS   = BD4^T [P2|Zp]        (PE partition reduce over (b,rr))
  S    = pS/Z + Brep ; OUT = squash(S)  (small [32,384] ops)
Iteration 1 (uniform c): S1 = psum_s1/IN + Brep via PSUM-accumulated
BD4^T IH_c matmuls during phase 1.

Toolchain constraint: EVERY engine instruction accepts at most ONE sync wait
at codegen.  Same-engine deps are free (program order / one monotonic sem per
engine); cross-engine fan-in is handled by absorb ops (tiny reads that
pre-observe a sem) and dummy matmuls on the PE.
"""

import numpy as np

B, IN, DIN = 32, 128, 256
CH, NC, DC = 32, 47, 64
NCP = 48          # padded Nc
NSH = 6           # capsules per core
NCORES = 8
NK = NSH * DC     # 384
EPS = 1e-7

_cache = {}


def _build_nc():
    import concourse.bass as bass
    import concourse.tile as tile
    from concourse import mybir
    from concourse.tile_rust import add_dep_helper

    f32 = mybir.dt.float32
    bf = mybir.dt.bfloat16
    nc = bass.Bass()

    # partition-major packed stream: xw[d, cd, 0:128]=xT, [128:512]=wT (bf16)
    xw = nc.dram_tensor("xw", [128, CH * 2, 512], bf, kind="ExternalInput")
    # consts: [bd4(0:32) | bd4t(rows0:32, 32:160) | brep(rows0:32, 160:544)]
    cst = nc.dram_tensor("cst", [128, 544], bf, kind="ExternalInput")
    out_d = nc.dram_tensor("out", [B, NK], f32, kind="ExternalOutput")

    ADD = mybir.AluOpType.add
    MULT = mybir.AluOpType.mult
    AX = mybir.AxisListType.X
    AF = mybir.ActivationFunctionType

    with tile.TileContext(nc) as tc:
        with (
            tc.tile_pool(name="singles", bufs=1) as singles,
            tc.tile_pool(name="work", bufs=1) as work,
            tc.tile_pool(name="small", bufs=2) as small,
            tc.tile_pool(name="ps_ih", bufs=3, space="PSUM") as ps_ih,
            tc.tile_pool(name="ps_s1", bufs=1, space="PSUM") as ps_s1,
            tc.tile_pool(name="ps_s", bufs=2, space="PSUM") as ps_s,
            tc.tile_pool(name="ps_rep", bufs=2, space="PSUM") as ps_rep,
        ):
            cst_t = singles.tile([128, 544], bf)
            c_dma = nc.sync.dma_start(out=cst_t[:], in_=cst[:])
            bd4_t = cst_t[:, 0:B]                 # [128, 32] bf16
            bd4t_t = cst_t[0:B, B:B + 128]        # [32, 128] bf16
            brep_t = cst_t[0:B, B + 128:B + 128 + NK]   # [32, 384] bf16
            eps_t = singles.tile([B, 1], f32)
            nc.vector.memset(eps_t[:], EPS)
            # DVE/ACT pre-observe the const-DMA sem
            dve_scratch = singles.tile([4, 8], bf)
            nc.vector.tensor_copy(dve_scratch[:2, 0:2], cst_t[:2, :2])
            act_scratch = singles.tile([4, 8], bf)
            nc.scalar.copy(act_scratch[:2, 0:2], cst_t[:2, :2])
            act_f32 = singles.tile([4, 2], f32)
            nc.scalar.activation(act_f32[:2, 0:2], act_scratch[:2, 0:2],
                                 AF.Exp)

            IH = singles.tile([128, CH, NK], bf)      # k-inner
            IHC = singles.tile([128, NK, CH + 1], bf)  # c-inner, pad stride 33
            STREAM = singles.tile([128, CH * 2, 512], bf)
            TMP = singles.tile([128, CH * NK], bf)    # mul product scratch
            U1 = singles.tile([128, 6144], bf)
            U2 = singles.tile([128, 3072], bf)
            SCR = singles.tile([128, NK + NSH], bf)   # [P2 | Zp]
            A2 = singles.tile([128, CH * NSH], f32)
            A3 = singles.tile([128, CH * NSH], f32)
            E = singles.tile([128, NSH * CH], bf)     # [p, (n, c)]
            OUTr = singles.tile([128, NK], bf)

            # Absorb the const-DMA sem into the PE clock (PE nop).
            last_dummy = nc.tensor.nop()
            add_dep_helper(last_dummy.ins, c_dma.ins, sync=True,
                           reason="absorb cst DMA sem into PE clock")

            psum_s1 = ps_s1.tile([B, NK], f32)

            # ---------------- phase 1: inputs_hat + iter-1 s ----------------
            s_dmas = []
            dma_splits = [(0, 2), (2, 22), (22, 43), (43, 64)]
            for gi, (lo, hi) in enumerate(dma_splits):
                dd = nc.sync.dma_start(
                    out=STREAM[:, lo:hi, :],
                    in_=xw[:, lo:hi, :],
                )
                if gi > 0:
                    add_dep_helper(dd.ins, s_dmas[0].ins, sync=True,
                                   reason="first chunk gets full DMA bandwidth")
                s_dmas.append(dd)
            # channel processing order (c, c+16) interleaved so the iter-1
            # tree-fold over c can start mid-phase (chunk j needs channels
            # 4j..4j+3 and 16+4j..19+4j = the first 8(j+1) positions)
            ch_order = []
            for t in range(CH // 2):
                ch_order += [t, t + CH // 2]
            U1s = U1[:].rearrange("p (n k c) -> p n k c", n=NSH, k=DC)

            def s1_mm(pos):
                nc.tensor.matmul(
                    psum_s1[:], bd4_t[:], IH[:, ch_order[pos], :],
                    start=(pos == 0), stop=(pos == CH - 1),
                    skip_group_check=True,
                )

            copy_last = []      # last psum reader per position
            for pos, c in enumerate(ch_order):
                if pos >= 3:
                    # absorb the psum-slot WAR tick into the PE clock
                    dmy = nc.tensor.nop()
                    add_dep_helper(dmy.ins, copy_last[pos - 3].ins, sync=True,
                                   reason="absorb psum WAR tick on PE")
                    last_dummy = dmy
                psum_ih = ps_ih.tile([128, NK], f32, tag="ih")
                for dc in range(2):
                    cd = pos * 2 + dc
                    mih = nc.tensor.matmul(
                        psum_ih[:], STREAM[:, cd, 0:128], STREAM[:, cd, 128:512],
                        start=(dc == 0), stop=(dc == 1),
                    )
                    if dc == 0:
                        add_dep_helper(mih.ins, last_dummy.ins, sync=False,
                                       reason="order dummy before matmul")
                # both copies for channel c on ONE engine (alternating) so
                # the WAR absorb needs a single sem
                if pos % 2 == 0:
                    nc.vector.tensor_copy(IH[:, c, :], psum_ih[:])
                    cl = nc.vector.tensor_copy(IHC[:, :, c], psum_ih[:])
                else:
                    nc.scalar.copy(IH[:, c, :], psum_ih[:])
                    cl = nc.scalar.copy(IHC[:, :, c], psum_ih[:])
                copy_last.append(cl)
                # iter-1 s accumulation, lagged 4 positions behind the copies
                if pos >= 4:
                    s1_mm(pos - 4)
            for pp in range(CH - 4, CH):
                s1_mm(pp)

            _absn = [0]

            def absorb(eng, src_ap):
                """Tiny copy on `eng` reading src_ap: pre-observes the
                producer's sem so the next real op keeps a single wait."""
                _absn[0] += 1
                scr = small.tile([2, 2], f32, tag="abs%d" % _absn[0])
                if eng == "v":
                    return nc.vector.tensor_copy(scr[:], src_ap)
                return nc.scalar.copy(scr[:], src_ap)

            def squash(S, it):
                """S: [B, NK] f32 sbuf tile -> OUT tile (bf16 it<3, f32 it=3)."""
                Ssq = work.tile([B, NK], f32, tag="Su")
                nc.vector.tensor_mul(Ssq[:], S[:], S[:])
                m2 = small.tile([B, NSH], f32, tag="m2")
                nc.vector.tensor_reduce(
                    m2[:], Ssq[:].rearrange("p (n k) -> p n k", n=NSH),
                    axis=AX, op=ADD,
                )
                d1 = small.tile([B, NSH], f32, tag="d1")
                nc.vector.tensor_scalar_add(d1[:], m2[:], 1.0)
                rd1 = small.tile([B, NSH], f32, tag="rd1")
                nc.vector.reciprocal(rd1[:], d1[:])
                absorb("s", m2[:2, :2])          # ACT clock <- m2 (DVE)
                # rsqrt(m2+eps) = exp(-0.5*ln(m2+eps)); ln+exp share one
                # ACT table set (no SQRT table thrash)
                ln_ = small.tile([B, NSH], f32, tag="ln")
                nc.scalar.activation(ln_[:], m2[:], AF.Ln, bias=eps_t[:])
                rsq = small.tile([B, NSH], f32, tag="rsq")
                nc.scalar.activation(rsq[:], ln_[:], AF.Exp, scale=-0.5)
                absorb("v", rsq[:2, :2])         # DVE clock <- rsq (ACT)
                t_ = small.tile([B, NSH], f32, tag="t")
                nc.vector.tensor_mul(t_[:], m2[:], rsq[:])
                g_ = small.tile([B, NSH], f32, tag="g")
                nc.vector.tensor_mul(g_[:], t_[:], rd1[:])
                OUT = work.tile([B, NK], f32 if it == 3 else bf,
                                tag="out%d" % it)
                nc.vector.tensor_mul(
                    OUT[:].rearrange("p (n k) -> p n k", n=NSH),
                    S[:].rearrange("p (n k) -> p n k", n=NSH),
                    g_[:].rearrange("p (n o) -> p n o", o=1)
                        .broadcast_to([B, NSH, DC]),
                )
                return OUT

            rep_mm_prev = [None]
            mm_last_ref = [None]

            def replicate(OUTb, it):
                """OUTb [B, NK] bf16 -> OUTr [128, NK] bf16 (row b -> 4b..4b+3)."""
                pr = ps_rep.tile([128, NK], f32, tag="rep")
                mm = nc.tensor.matmul(pr[:], bd4t_t[:], OUTb[:],
                                      start=True, stop=True)
                rep_mm_prev[0] = mm
                cp = nc.vector.tensor_copy(OUTr[:], pr[:])
                return mm, cp

            # ---------------- iter 1 ----------------
            S1 = work.tile([B, NK], f32, tag="S")
            nc.vector.scalar_tensor_tensor(
                out=S1[:], in0=psum_s1[:], scalar=1.0 / IN, in1=brep_t[:],
                op0=MULT, op1=ADD,
            )
            OUT1 = squash(S1, 1)
            rep_mm, rep_cp = replicate(OUT1, 1)

            TMPk = TMP[:].rearrange("p (c n k) -> p c n k", c=CH, n=NSH)
            TMPc = TMP[:].rearrange("p (n k c) -> p n k c", n=NSH, k=DC)
            U1k = U1[:].rearrange("p (c n k) -> p c n k", c=CH, n=NSH)
            U2k = U2[:].rearrange("p (c n k) -> p c n k", c=CH, n=NSH)
            U1c = U1[:].rearrange("p (n k c) -> p n k c", n=NSH, k=DC)
            U2c = U2[:].rearrange("p (n k c) -> p n k c", n=NSH, k=DC)

            for it in (2, 3):
                # ---- a-step: TMP = IH * OUTr ; A = tree-fold k ----
                nc.vector.tensor_mul(
                    TMP[:].rearrange("p (c nk) -> p c nk", c=CH),
                    IH[:].rearrange("p c nk -> p c nk"),
                    OUTr[:].rearrange("p (o nk) -> p o nk", o=1)
                          .broadcast_to([128, CH, NK]),
                )
                nc.vector.tensor_add(U1k[:, :, :, 0:32], TMPk[:, :, :, 0:32],
                                     TMPk[:, :, :, 32:64])
                nc.vector.tensor_add(U2k[:, :, :, 0:16], U1k[:, :, :, 0:16],
                                     U1k[:, :, :, 16:32])
                nc.vector.tensor_add(U1k[:, :, :, 0:8], U2k[:, :, :, 0:8],
                                     U2k[:, :, :, 8:16])
                nc.vector.tensor_add(U2k[:, :, :, 0:4], U1k[:, :, :, 0:4],
                                     U1k[:, :, :, 4:8])
                nc.vector.tensor_add(U1k[:, :, :, 0:2], U2k[:, :, :, 0:2],
                                     U2k[:, :, :, 2:4])
                At = A2 if it == 2 else A3
                nc.vector.tensor_add(
                    At[:].rearrange("p (c n o) -> p c n o", c=CH, o=1),
                    U1k[:, :, :, 0:1], U1k[:, :, :, 1:2],
                )
                if it == 2:
                    BL = A2
                else:
                    BL = A3
                    nc.vector.tensor_add(A3[:], A3[:], A2[:])
                # ---- E = exp(BL), transposed write to [p, (n, c)] ----
                absorb("s", At[:2, :2])         # ACT clock <- tree (DVE)
                nc.scalar.activation(
                    E[:].rearrange("p (n c) -> p c n", n=NSH),
                    BL[:].rearrange("p (c n) -> p c n", c=CH),
                    AF.Exp,
                )
                # ---- Zp = sum_c E -> SCR[384:390] ----
                absorb("v", E[:2, :2])          # DVE clock <- E (ACT)
                with nc.allow_low_precision(reason="Z normalizer, positive sum"):
                    nc.vector.tensor_reduce(
                        SCR[:, NK:NK + NSH],
                        E[:].rearrange("p (n c) -> p n c", n=NSH),
                        axis=AX, op=ADD,
                    )
                # ---- s-step: TMP2 = IHC * E ; P2 = tree-fold c ----
                nc.vector.tensor_mul(
                    TMPc,
                    IHC[:, :, 0:CH]
                       .rearrange("p (n k) c -> p n k c", n=NSH),
                    E[:].rearrange("p (n o c) -> p n o c", n=NSH, o=1)
                       .broadcast_to([128, NSH, DC, CH]),
                )
                nc.vector.tensor_add(U1c[:, :, :, 0:16], TMPc[:, :, :, 0:16],
                                     TMPc[:, :, :, 16:32])
                nc.vector.tensor_add(U2c[:, :, :, 0:8], U1c[:, :, :, 0:8],
                                     U1c[:, :, :, 8:16])
                nc.vector.tensor_add(U1c[:, :, :, 0:4], U2c[:, :, :, 0:4],
                                     U2c[:, :, :, 4:8])
                nc.vector.tensor_add(U2c[:, :, :, 0:2], U1c[:, :, :, 0:2],
                                     U1c[:, :, :, 2:4])
                nc.vector.tensor_add(
                    SCR[:, 0:NK].rearrange("p (n k o) -> p n k o", n=NSH, o=1),
                    U2c[:, :, :, 0:1], U2c[:, :, :, 1:2])
                # ---- pS = BD4^T [P2|Zp] ----
                pS = ps_s.tile([B, NK + NSH], f32, tag="pS")
                mm_last = nc.tensor.matmul(pS[:], bd4_t[:], SCR[:],
                                           start=True, stop=True)
                mm_last_ref[0] = mm_last
                # ---- S = pS/Z + brep ----
                absorb("v", pS[:2, :2])         # DVE clock <- pS (PE)
                Rz = small.tile([B, NSH], f32, tag="Rz")
                nc.vector.reciprocal(Rz[:], pS[:, NK:NK + NSH])
                Su = work.tile([B, NK], f32, tag="Su2")
                nc.vector.tensor_mul(
                    Su[:].rearrange("p (n k) -> p n k", n=NSH),
                    pS[:, 0:NK].rearrange("p (n k) -> p n k", n=NSH),
                    Rz[:].rearrange("p (n o) -> p n o", o=1)
                        .broadcast_to([B, NSH, DC]),
                )
                S = work.tile([B, NK], f32, tag="S")
                nc.vector.tensor_add(S[:], Su[:], brep_t[:])
                OUT = squash(S, it)
                if it < 3:
                    rep_mm, rep_cp = replicate(OUT, it)
                else:
                    # absorb stream/cst DMA queue sems into SYNC first so the
                    # out-DMA's queue-reuse wait dedups to a single sem
                    for fin in (c_dma, *s_dmas):
                        fnop = nc.sync.nop()
                        add_dep_helper(fnop.ins, fin.ins, sync=True,
                                       reason="absorb DMA sem for queue reuse")
                    o_dma = nc.sync.dma_start(out=out_d[:], in_=OUT[:])
                    f_scr = small.tile([2, 4], f32, tag="fin")
                    f_act = nc.scalar.copy(f_scr[:, 0:2], OUT[:2, :2])
                    f_dve = nc.vector.tensor_copy(f_scr[:, 2:4], OUT[:2, :2])
                    for fin in (mm_last, f_act, f_dve, o_dma):
                        fnop = nc.sync.nop()
                        add_dep_helper(fnop.ins, fin.ins, sync=True,
                                       reason="absorb final sem for tail drain")

    return nc


def _pack_inputs(inputs, W, B_param):
    """Host-side shard + relayout. Returns list of 8 in_maps."""
    import ml_dtypes
    bf16 = ml_dtypes.bfloat16
    inputs = np.ascontiguousarray(inputs, dtype=np.float32)
    W = np.ascontiguousarray(W, dtype=np.float32)
    B_param = np.ascontiguousarray(B_param, dtype=np.float32)

    Wp = np.zeros((CH, NCP, DC, DIN), dtype=np.float32)
    Wp[:, :NC] = W
    Bp = np.zeros((NCP, DC), dtype=np.float32)
    Bp[:NC] = B_param

    # xt[(c,dc), dd, (b,rr)] = x[b, 4c+rr, 128dc+dd]
    x4 = inputs.reshape(B, CH, 4, 2, 128)           # b, c, rr, dc, dd
    xt = x4.transpose(1, 3, 4, 0, 2).reshape(CH * 2, 128, 128)
    bd4 = np.zeros((128, B), dtype=np.float32)
    bd4[np.arange(128), np.arange(128) // 4] = 1.0
    bd4t = bd4.T

    in_maps = []
    for core in range(NCORES):
        sl = slice(core * NSH, (core + 1) * NSH)
        Wc = Wp[:, sl]                               # c, n, k, d
        w5 = Wc.reshape(CH, NSH, DC, 2, 128)         # c n k dc dd
        wtc = w5.transpose(0, 3, 4, 1, 2).reshape(CH * 2, 128, NK)
        cstc = np.zeros((128, 544), dtype=np.float32)
        cstc[:, 0:B] = bd4
        cstc[0:B, B:B + 128] = bd4t
        cstc[0:B, B + 128:B + 128 + NK] = np.broadcast_to(
            Bp[sl].reshape(1, NK), (B, NK))
        xwc = np.concatenate([xt, wtc], axis=2)      # [64, 128, 512]
        # reorder chunks to the kernel's interleaved channel order
        ch_order = []
        for t in range(CH // 2):
            ch_order += [t, t + CH // 2]
        perm = np.zeros(CH * 2, dtype=np.int64)
        for p_, c_ in enumerate(ch_order):
            perm[2 * p_] = 2 * c_
            perm[2 * p_ + 1] = 2 * c_ + 1
        xwc = xwc[perm]
        # partition-major: [d, cd, 512] for 32KB-contiguous DMA runs
        xwc = np.ascontiguousarray(xwc.transpose(1, 0, 2)).astype(bf16)
        in_maps.append(dict(xw=xwc, cst=cstc.astype(bf16)))
    return in_maps


def _run(inputs, W, B_param, trace=False):
    from concourse.bass_utils import run_bass_kernel_spmd

    if "nc" not in _cache:
        _cache["nc"] = _build_nc()
    nc = _cache["nc"]
    in_maps = _pack_inputs(inputs, W, B_param)
    res = run_bass_kernel_spmd(nc, in_maps, core_ids=list(range(NCORES)),
                               trace=trace)
    outs = [r["out"].reshape(B, NSH, DC) for r in res.results]
    full = np.concatenate(outs, axis=1)[:, :NC, :]
    return np.ascontiguousarray(full.astype(np.float32)), res


def kernel(inputs, W, B_param):
    out, _ = _run(inputs, W, B_param, trace=False)
    return out
